# revision 106
# baseline (speedup 1.0000x reference)
"""GroupedQueryAttention Trainium2 kernel (v4, bf16).

Problem: B=2, T=1024, M=2048, D=128, G=4 kv-groups, H=4 heads/group.
Sharding: core c = 4*b + g (batch x kv-group), no collectives; host sums
the G partial outputs per batch.

All matmul operands are bf16 (norm rel err ~5.5e-3 vs the 2e-2 gate);
PSUM accumulation stays fp32. Host pre-arranges x/weights into
partition-major [128, ...] layouts so every DMA moves >=512B contiguous
runs at the full 360GB/s device rate.

Schedule (cost-model-guided; predicted ~107.5us/core vs 177us for the
f32r v2 baseline):
  phase 1: x/wk/wq0/wv stream in k-consumption order on the two HWDGE
    queues (wo/wq2-3 behind them, consts on SWDGE). k/q0/q1 projections
    run k-outer while x streams (q1 rides the projB banks - its
    PSUM-resident matmuls dilute the DMA demand rate below the 360GB/s
    device limit); v is projected t-outer afterwards directly in [t, D]
    layout (lhsT = x tile) - each PSUM bank has exactly one live
    accumulation region because a start=True matmul wipes the whole
    bank, not just its output range. RoPE: psum is read once (DVE copy),
    cos-mul on DVE from the copy, half-swap via SBUF-SBUF DMA, sin-mul
    and add on Pool so a DMA wait never blocks the DVE queue. q2/q3
    follow per-head with double-buffered half tiles.
  phase 2: scores per (h, j-block) in split lo/hi [128,512] PSUM tiles
    (causal mask accumulated on the diagonal via a bf16 PE matmul),
    emitted 2 blocks ahead; exp chunks are emitted with the scores so
    the ACT engine runs a full iteration ahead of the PE's AV matmuls.
    den rides a 1-wide ones matmul per chunk into per-half single-bank
    tiles; o/den accumulate j=0..3 (lo) and j=0..7 (hi) per head and
    normalization (DVE reciprocal -> Pool partition_broadcast -> Pool
    multiply into oT) overlaps the j loop on otherwise-idle engines. The h3-hi normalization is deferred into phase
    3 so the PE never waits on the reciprocal chain at the boundary.
  phase 3: out-projection in [128,512] PSUM chunks (bufs=4), 4 h-matmuls
    per chunk, evictions alternating DVE/ACT and DMAs alternating
    sync/scalar queues so the final drain is one chunk deep.
"""

import sys
import numpy as np

sys.path.insert(0, "/opt/trn_rl_repo")

import concourse.bass as bass  # noqa: E402
import concourse.bass_isa as bass_isa  # noqa: E402
import concourse.tile as tile  # noqa: E402
from concourse import bacc, mybir  # noqa: E402
from concourse.bass_utils import run_bass_kernel_spmd  # noqa: E402
from contextlib import ExitStack  # noqa: E402

F32 = mybir.dt.float32
F32R = mybir.dt.float32r
BF16 = mybir.dt.bfloat16

B, T, M, D, G, H = 2, 1024, 2048, 128, 4, 4
KT = M // 128   # 16 contraction tiles
TT = T // 128   # 8 sequence tiles
ROPE_THETA = 10000.0
NEG = -1.0e30

# x DMA chunk sizes in k-tiles (first two small to unblock the PE, last
# ones large to cut HWDGE per-DMA overhead)
XCH = [1, 1, 2, 2, 2, 4, 4]
XOF = [sum(XCH[:i]) for i in range(len(XCH))]   # chunk k-offsets

_CACHE = {}
_MARKS = []
_DEBUG = False


def _mark(nc, label):
    _MARKS.append((label, int(nc.get_next_instruction_name().split("-")[1])))


def _xck(k):
    """x chunk index and sub-index for k-tile k."""
    for i in range(len(XCH) - 1, -1, -1):
        if k >= XOF[i]:
            return i, k - XOF[i]
    raise ValueError(k)


def _chunks(i0):
    """Column ranges covering [i0, T), split at the 512 PSUM bank edge."""
    if i0 < 512:
        return [(i0, 512), (512, 1024)]
    return [(i0, 1024)]


def _build_program():
    nc = bacc.Bacc("TRN2", target_bir_lowering=False, debug=False, num_devices=8)

    xT_d = nc.dram_tensor("xT", [128, KT, T], BF16, kind="ExternalInput").ap()
    wq_d = nc.dram_tensor("wq", [H, 128, KT, D], BF16, kind="ExternalInput").ap()
    wk_d = nc.dram_tensor("wk", [128, KT, D], BF16, kind="ExternalInput").ap()
    wv_d = nc.dram_tensor("wv", [128, KT, D], BF16, kind="ExternalInput").ap()
    wo_d = nc.dram_tensor("wo", [128, H, M], BF16, kind="ExternalInput").ap()
    cc_d = nc.dram_tensor("cc", [128, T], F32, kind="ExternalInput").ap()
    ss_d = nc.dram_tensor("ss", [128, T], F32, kind="ExternalInput").ap()
    maddT_d = nc.dram_tensor("maddT", [128, 128], BF16, kind="ExternalInput").ap()
    identr_d = nc.dram_tensor("identr", [128, 128], BF16, kind="ExternalInput").ap()
    onec_d = nc.dram_tensor("onec", [128, 1], BF16, kind="ExternalInput").ap()
    oner_d = nc.dram_tensor("oner", [1, 128], F32R, kind="ExternalInput").ap()
    r_d = nc.dram_tensor("r", [T, M], BF16, kind="ExternalOutput").ap()
    if _DEBUG:
        dbg_q_d = nc.dram_tensor("dbg_q", [128, H, T], BF16, kind="ExternalOutput").ap()
        dbg_k_d = nc.dram_tensor("dbg_k", [128, T], BF16, kind="ExternalOutput").ap()
        dbg_v_d = nc.dram_tensor("dbg_v", [128, TT, 128], BF16, kind="ExternalOutput").ap()
        dbg_o_d = nc.dram_tensor("dbg_o", [128, H, T], BF16, kind="ExternalOutput").ap()
        dbg_cc_d = nc.dram_tensor("dbg_cc", [128, T], F32, kind="ExternalOutput").ap()

    def wsl8(d, c):
        return d[:, 8 * c:8 * c + 8, :]

    with tile.TileContext(nc) as tc, ExitStack() as ctx:
        const = ctx.enter_context(tc.tile_pool(name="const", bufs=1))
        persist = ctx.enter_context(tc.tile_pool(name="persist", bufs=1))

        cc_sb = const.tile([128, T], F32)
        ss_sb = const.tile([128, T], F32)
        maddT_sb = const.tile([128, 128], BF16)
        identr_sb = const.tile([128, 128], BF16)
        onec_sb = const.tile([128, 1], BF16)
        oner_sb = const.tile([1, 128], F32R)

        qT = persist.tile([128, H, T], BF16)      # roped, scaled q heads
        kT = persist.tile([128, T], BF16)         # roped, scaled k
        vsb = persist.tile([128, TT, 128], BF16)  # v as [t, D] tiles
        oT = persist.tile([128, H, T], BF16)      # normalized attn outputs
        wo_sb = persist.tile([128, H, M], BF16)

        # ---------------- Phase 1: projections + rope ----------------
        p1 = ExitStack()
        xpool = p1.enter_context(tc.tile_pool(name="xp", bufs=1))
        wpool = p1.enter_context(tc.tile_pool(name="wp", bufs=1))
        ppool = p1.enter_context(tc.tile_pool(name="proj_ps", bufs=1, space="PSUM"))
        epool = p1.enter_context(tc.tile_pool(name="ev", bufs=4))
        rtmp = p1.enter_context(tc.tile_pool(name="rope", bufs=6))

        # --- DMA emission: critical-path loads on the two HWDGE queues
        # (sync=SP, scalar=ACT) in consumption order; consts on the gpsimd
        # SWDGE queue; wo at the back so it can't head-of-line block.
        WCH = [2, 6, 8]
        WOF = [0, 2, 8]

        def _wck(k):
            ci = 2 if k >= 8 else (1 if k >= 2 else 0)
            return ci, k - WOF[ci]

        wk_c = [wpool.tile([128, WCH[c], D], BF16, tag=f"wk{c}", name=f"wk_c{c}")
                for c in range(3)]
        wq0_c = [wpool.tile([128, WCH[c], D], BF16, tag=f"wq0{c}", name=f"wq0_c{c}")
                 for c in range(3)]
        wv_c = [wpool.tile([128, WCH[c], D], BF16, tag=f"wv{c}", name=f"wv_c{c}")
                for c in range(3)]
        x_t = [xpool.tile([128, XCH[c], T], BF16, tag=f"x{c}", name=f"x_t{c}")
               for c in range(len(XCH))]
        wq1_c = [wpool.tile([128, WCH[c], D], BF16, tag=f"wq1{c}",
                            name=f"wq1_c{c}") for c in range(3)]
        wqh_c = {h: [wpool.tile([128, 8, D], BF16, tag=f"wq{h}{c}",
                                name=f"wq{h}_c{c}")
                     for c in range(2)] for h in (2, 3)}

        def xdma(c):
            nc.sync.dma_start(out=x_t[c], in_=xT_d[:, XOF[c]:XOF[c] + XCH[c], :])

        def wdma(w_c, d, c):
            nc.scalar.dma_start(out=w_c[c], in_=d[:, WOF[c]:WOF[c] + WCH[c], :])

        wdma(wk_c, wk_d, 0)
        xdma(0)
        wdma(wq0_c, wq_d[0], 0)
        xdma(1)
        wdma(wq1_c, wq_d[1], 0)
        wdma(wv_c, wv_d, 0)
        xdma(2)
        wdma(wk_c, wk_d, 1)
        xdma(3)
        wdma(wq0_c, wq_d[0], 1)
        wdma(wq1_c, wq_d[1], 1)
        wdma(wv_c, wv_d, 1)
        xdma(4)
        wdma(wk_c, wk_d, 2)
        wdma(wq0_c, wq_d[0], 2)
        xdma(5)
        wdma(wq1_c, wq_d[1], 2)
        wdma(wv_c, wv_d, 2)
        xdma(6)
        nc.sync.dma_start(out=cc_sb, in_=cc_d)
        nc.sync.dma_start(out=ss_sb, in_=ss_d)
        for sb, d in ((maddT_sb, maddT_d), (identr_sb, identr_d),
                      (onec_sb, onec_d), (oner_sb, oner_d)):
            nc.gpsimd.dma_start(out=sb, in_=d)
        nc.scalar.dma_start(out=wqh_c[2][0], in_=wsl8(wq_d[2], 0))
        nc.sync.dma_start(out=wqh_c[2][1], in_=wsl8(wq_d[2], 1))
        nc.scalar.dma_start(out=wqh_c[3][0], in_=wsl8(wq_d[3], 0))
        nc.sync.dma_start(out=wqh_c[3][1], in_=wsl8(wq_d[3], 1))
        for mc in range(4):
            eng = nc.scalar if mc % 2 == 0 else nc.sync
            eng.dma_start(out=wo_sb[:, :, 512 * mc:512 * mc + 512],
                          in_=wo_d[:, :, 512 * mc:512 * mc + 512])

        # --- pass A: kT, qT0, qT1 interleaved k-outer (q1 rides the two
        # projB banks; its PSUM-resident matmuls dilute the DMA demand
        # rate below the 360GB/s device limit) ---
        _mark(nc, "passA")
        ps_v = [ppool.tile([128, 128], F32, tag=f"pv{i}", name=f"ps_v{i}")
                for i in range(2)]
        ps_k = ppool.tile([128, T], F32, tag="pk", name="ps_k")
        ps_q0 = ppool.tile([128, T], F32, tag="pq0", name="ps_q0")
        ps_q1 = [ppool.tile([128, 512], F32, tag="projB",
                            name=f"ps_q1_{nch}", bufs=2) for nch in range(2)]

        def x_ap(k):
            ci, si = _xck(k)
            return x_t[ci][:, si, :]

        for k in range(KT):
            ci, si = _wck(k)
            for nch in range(2):
                nc.tensor.matmul(
                    ps_k[:, nch * 512:(nch + 1) * 512],
                    lhsT=wk_c[ci][:, si, :],
                    rhs=x_ap(k)[:, nch * 512:(nch + 1) * 512],
                    start=(k == 0), stop=(k == KT - 1))
            for nch in range(2):
                nc.tensor.matmul(
                    ps_q0[:, nch * 512:(nch + 1) * 512],
                    lhsT=wq0_c[ci][:, si, :],
                    rhs=x_ap(k)[:, nch * 512:(nch + 1) * 512],
                    start=(k == 0), stop=(k == KT - 1))
            for nch in range(2):
                nc.tensor.matmul(
                    ps_q1[nch],
                    lhsT=wq1_c[ci][:, si, :],
                    rhs=x_ap(k)[:, nch * 512:(nch + 1) * 512],
                    start=(k == 0), stop=(k == KT - 1))

        def rope_tail(ps, out_ap, c0, n, tag, ev_eng=None):
            """Finish rope: psum f32 [D, n] -> out_ap bf16 (cols c0:c0+n).
            The psum has a single reader (ev, first on DVE) so the
            projection buffer frees after one op; everything else reads
            the SBUF copy. The swap-dependent tail runs on Pool so a DMA
            wait never blocks the DVE queue."""
            ev = epool.tile([128, n], F32, tag="ev", name=f"ev_{tag}")
            if ev_eng is None:
                nc.vector.tensor_copy(out=ev, in_=ps)
            else:
                ev_eng.copy(ev, ps)
            a_t = rtmp.tile([128, n], F32, tag="ra", name=f"ra_{tag}")
            nc.vector.tensor_mul(a_t, ev, cc_sb[:, c0:c0 + n])
            swp = rtmp.tile([128, n], F32, tag="swp", name=f"swp_{tag}")
            nc.sync.dma_start(out=swp[0:64, :], in_=ev[64:128, :])
            nc.sync.dma_start(out=swp[64:128, :], in_=ev[0:64, :])
            b_t = rtmp.tile([128, n], F32, tag="rb", name=f"rb_{tag}")
            nc.gpsimd.tensor_mul(b_t, swp, ss_sb[:, c0:c0 + n])
            nc.gpsimd.tensor_add(out_ap, a_t, b_t)

        # --- v: t-outer, k-inner (x is resident by now); one live
        # accumulation region per PSUM bank (start=True wipes the bank)
        _mark(nc, "ropeA")
        # 512-col halves keep every Pool op ~1us and rotate the rope
        # buffers quickly
        for nch in range(2):
            c0 = nch * 512
            rope_tail(ps_q1[nch], qT[:, 1, c0:c0 + 512], c0, 512,
                      f"q1{nch}")
            rope_tail(ps_k[:, c0:c0 + 512], kT[:, c0:c0 + 512], c0, 512,
                      f"k{nch}")
            rope_tail(ps_q0[:, c0:c0 + 512], qT[:, 0, c0:c0 + 512], c0, 512,
                      f"q0{nch}")

        _mark(nc, "v_proj")
        for t in range(TT):
            ps = ps_v[t % 2]
            for k in range(KT):
                ci, si = _wck(k)
                nc.tensor.matmul(
                    ps,
                    lhsT=x_ap(k)[:, t * 128:(t + 1) * 128],
                    rhs=wv_c[ci][:, si, :],
                    start=(k == 0), stop=(k == KT - 1))
            nc.scalar.copy(vsb[:, t, :], ps)

        # --- pass B: qT2..3, half tiles ---
        for h in (2, 3):
            _mark(nc, f"qT{h}_proj")
            for nch in range(2):
                psB = ppool.tile([128, 512], F32, tag="projB",
                                 name=f"ps_q{h}_{nch}", bufs=2)
                for k in range(KT):
                    nc.tensor.matmul(
                        psB, lhsT=wqh_c[h][k // 8][:, k % 8, :],
                        rhs=x_ap(k)[:, nch * 512:(nch + 1) * 512],
                        start=(k == 0), stop=(k == KT - 1))
                rope_tail(psB, qT[:, h, nch * 512:(nch + 1) * 512],
                          nch * 512, 512, f"q{h}_{nch}")
        p1.close()

        # ---------------- Phase 2: attention ----------------
        # opool/dpool/misc/ptpool outlive phase 2: the h3-hi normalization
        # is emitted in the middle of phase 3.
        opool = ctx.enter_context(tc.tile_pool(name="o_ps", bufs=2, space="PSUM"))
        dpool = ctx.enter_context(tc.tile_pool(name="d_ps", bufs=2, space="PSUM"))
        ptpool = ctx.enter_context(tc.tile_pool(name="pt", bufs=4))
        misc = ctx.enter_context(tc.tile_pool(name="amisc", bufs=2))

        p2 = ExitStack()
        spool = p2.enter_context(tc.tile_pool(name="s_ps", bufs=2, space="PSUM"))

        pairs = [(h, j) for h in range(H) for j in range(TT)]
        s_tiles = {}

        def emit_s(h, j):
            """Score block + additive causal mask on the diag (all PE).
            Separate lo/hi [128,512] tiles: lo (with mask) closes first so
            the lo-half exp can start early; j>=4 uses only a hi tile."""
            i0 = j * 128
            s_lo = None
            if j < 4:
                s_lo = spool.tile([128, 512], F32, tag="slo",
                                  name=f"slo_{h}_{j}", bufs=1)
                nc.tensor.matmul(s_lo[:, i0:512], lhsT=kT[:, i0:i0 + 128],
                                 rhs=qT[:, h, i0:512], start=True, stop=False)
                nc.tensor.matmul(s_lo[:, i0:i0 + 128], lhsT=maddT_sb,
                                 rhs=identr_sb, start=False, stop=True)
                s_hi = spool.tile([128, 512], F32, tag="shi",
                                  name=f"shi_{h}_{j}", bufs=3)
                nc.tensor.matmul(s_hi, lhsT=kT[:, i0:i0 + 128],
                                 rhs=qT[:, h, 512:1024], start=True, stop=True)
            else:
                s_hi = spool.tile([128, 512], F32, tag="shi",
                                  name=f"shi_{h}_{j}", bufs=3)
                nc.tensor.matmul(s_hi[:, i0 - 512:512], lhsT=kT[:, i0:i0 + 128],
                                 rhs=qT[:, h, i0:1024], start=True, stop=False)
                nc.tensor.matmul(s_hi[:, i0 - 512:i0 - 384], lhsT=maddT_sb,
                                 rhs=identr_sb, start=False, stop=True)
            # exp emitted here (not at consume time) so the ACT engine
            # runs a full iteration ahead of the PE's AV matmuls; one op
            # per 512-chunk tile keeps per-op overhead minimal
            p_sb = ptpool.tile([128, T], BF16, tag="pT", name=f"p_{h}_{j}")
            if s_lo is not None:
                nc.scalar.activation(out=p_sb[:, i0:512], in_=s_lo[:, i0:512],
                                     func=mybir.ActivationFunctionType.Exp)
                nc.scalar.activation(out=p_sb[:, 512:1024], in_=s_hi,
                                     func=mybir.ActivationFunctionType.Exp)
            else:
                nc.scalar.activation(out=p_sb[:, i0:1024],
                                     in_=s_hi[:, i0 - 512:512],
                                     func=mybir.ActivationFunctionType.Exp)
            s_tiles[(h, j)] = p_sb

        den_t = {}
        o_t = {}
        o_ev = {}

        def get_half(pool, store, h, half, shape, tag):
            if (h, half) not in store:
                store[(h, half)] = pool.tile(shape, F32, tag=tag,
                                             name=f"{tag}_{h}_{half}")
            return store[(h, half)]

        def get_den(h, half):
            # one [1,512] tile per half, each in its own PSUM bank: a
            # start=True wipes the whole bank, so lo/hi must not share one
            if (h, half) not in den_t:
                den_t[(h, half)] = dpool.tile(
                    [1, 512], F32, tag=f"den{half}",
                    name=f"den_{h}_{half}", bufs=1)
            return den_t[(h, half)]

        def norm_half(h, half):
            """reciprocal -> f32r -> K=1 broadcast -> multiply into oT."""
            lo, hi = (0, 512) if half == 0 else (512, 1024)
            den = den_t[(h, half)]
            dinv = misc.tile([1, 512], F32, tag="dinv", name=f"dinv_{h}_{half}")
            nc.vector.reciprocal(dinv, den)
            bc_sb = misc.tile([128, 512], F32, tag="bc", name=f"bc_{h}_{half}")
            nc.gpsimd.partition_broadcast(bc_sb, dinv)
            nc.gpsimd.tensor_mul(oT[:, h, lo:hi], o_ev.pop((h, half)), bc_sb)

        emitted = set()

        def ensure_emitted(idx):
            if 0 <= idx < len(pairs) and idx not in emitted:
                emitted.add(idx)
                emit_s(*pairs[idx])

        ensure_emitted(0)
        for idx, (h, j) in enumerate(pairs):
            _mark(nc, f"att_h{h}_j{j}")
            i0 = j * 128
            ensure_emitted(idx + 1)
            ensure_emitted(idx + 2)
            # prefetch the next head's first score tile one slot earlier:
            # its exp then clears ACT before the head-boundary AV needs it
            if idx + 3 < len(pairs) and pairs[idx + 3][1] == 0:
                ensure_emitted(idx + 3)
            p_sb = s_tiles.pop((h, j))
            for (cs, ce) in _chunks(i0):
                half = 0 if ce <= 512 else 1
                off = 0 if half == 0 else 512
                den = get_den(h, half)
                o_ps = get_half(opool, o_t, h, half, [128, 512], "o")
                last = (j == 3) if half == 0 else (j == TT - 1)
                nc.tensor.matmul(den[:, cs - off:ce - off], lhsT=onec_sb,
                                 rhs=p_sb[:, cs:ce],
                                 start=(j == 0), stop=last)
                nc.tensor.matmul(o_ps[:, cs - off:ce - off], lhsT=vsb[:, j, :],
                                 rhs=p_sb[:, cs:ce],
                                 start=(j == 0), stop=last)
                if last:
                    ev = misc.tile([128, 512], F32, tag=f"oev{half}",
                                   name=f"oev_{h}_{half}")
                    nc.vector.tensor_copy(out=ev, in_=o_t.pop((h, half)))
                    o_ev[(h, half)] = ev
            if j == 4:
                norm_half(h, 0)
            if j == TT - 1 and h != H - 1:
                norm_half(h, 1)
        p2.close()

        # ---------------- Phase 3: output projection ----------------
        # [128,512] PSUM chunks, bufs=4: 4 h-matmuls per chunk, evict
        # alternating DVE/ACT, DMA alternating sync/scalar queues.
        rpool = ctx.enter_context(tc.tile_pool(name="r_ps", bufs=4, space="PSUM"))
        rsb_pool = ctx.enter_context(tc.tile_pool(name="rsb", bufs=4))
        chunks3 = [(t, mc) for t in range(TT) for mc in range(4)]
        for gi, (t, mc) in enumerate(chunks3):
            if t == 4 and mc == 0:
                _mark(nc, "norm_h3_hi")
                norm_half(H - 1, 1)
            _mark(nc, f"rproj_t{t}_{mc}")
            last = (gi == len(chunks3) - 1)
            # the final chunk works in two 256-col halves on both engine/
            # queue pairs so the post-matmul drain is as short as possible
            subs = ((0, 512),)
            _ = last
            r_ps = rpool.tile([128, 512], F32, tag="r", name=f"r_{t}_{mc}")
            m0 = mc * 512
            for (a, b) in subs:
                for h in range(H):
                    nc.tensor.matmul(
                        r_ps[:, a:b],
                        lhsT=oT[:, h, t * 128:(t + 1) * 128],
                        rhs=wo_sb[:, h, m0 + a:m0 + b],
                        start=(h == 0), stop=(h == H - 1))
                r_sb = rsb_pool.tile([128, b - a], BF16, tag=f"rsb{(gi + a) % 4}",
                                     name=f"rsb_{t}_{mc}_{a}")
                if (gi + a) % 2 == 0:
                    nc.vector.tensor_copy(out=r_sb, in_=r_ps[:, a:b])
                    eng = nc.sync
                else:
                    nc.scalar.copy(r_sb, r_ps[:, a:b])
                    eng = nc.scalar
                eng.dma_start(out=r_d[t * 128:(t + 1) * 128,
                                      m0 + a:m0 + b],
                              in_=r_sb)

        if _DEBUG:
            nc.sync.dma_start(out=dbg_q_d, in_=qT)
            nc.sync.dma_start(out=dbg_k_d, in_=kT)
            nc.sync.dma_start(out=dbg_v_d, in_=vsb)
            nc.sync.dma_start(out=dbg_o_d, in_=oT)
            nc.sync.dma_start(out=dbg_cc_d, in_=cc_sb)

    nc.compile()
    return nc


def _host_tables():
    half = D // 2
    qk = np.float32(D ** (-0.25))
    pos = np.arange(T, dtype=np.float32)[:, None]
    freqs = np.power(np.float32(ROPE_THETA),
                     -np.arange(half, dtype=np.float32) / np.float32(half))[None, :]
    ang = pos * freqs                      # [T, 64] fp32
    cosT = (np.cos(ang) * qk).astype(np.float32).T.copy()   # [64, T]
    sinT = (np.sin(ang) * qk).astype(np.float32).T.copy()
    cc = np.concatenate([cosT, cosT], axis=0)               # [128, T]
    ss = np.concatenate([-sinT, sinT], axis=0)
    ii = np.arange(128)
    maddT = np.where(ii[None, :] > ii[:, None], np.float32(NEG),
                     np.float32(0.0)).astype(np.float32)     # [i, j] upper strict
    identr = np.eye(128, dtype=np.float32)
    return cc, ss, maddT, identr


LAST_RESULTS = None


def _bf16(a):
    import ml_dtypes
    return np.ascontiguousarray(a).astype(ml_dtypes.bfloat16)


def _arrange_w(w):
    """[M, D] -> [128, KT, D] partition-major."""
    return w.reshape(KT, 128, D).transpose(1, 0, 2)


def kernel(x, w_aq, w_ak, w_av, w_ao, _trace=False):
    global LAST_RESULTS
    if "nc" not in _CACHE:
        _CACHE["nc"] = _build_program()
    nc = _CACHE["nc"]

    cc, ss, maddT, identr = _host_tables()
    onec = np.ones((128, 1), dtype=np.float32)
    oner = np.ones((1, 128), dtype=np.float32)

    xT = [_bf16(x[b].T.reshape(KT, 128, T).transpose(1, 0, 2)) for b in range(B)]
    in_maps = []
    for c in range(8):
        b, g = divmod(c, G)
        wq = np.stack([_arrange_w(w_aq[g, h]) for h in range(H)], axis=0)
        in_maps.append({
            "xT": xT[b],
            "wq": _bf16(wq),
            "wk": _bf16(_arrange_w(w_ak[g])),
            "wv": _bf16(_arrange_w(w_av[g])),
            "wo": _bf16(w_ao[g].transpose(1, 0, 2)),
            "cc": cc, "ss": ss,
            "maddT": _bf16(maddT), "identr": _bf16(identr),
            "onec": _bf16(onec), "oner": oner,
        })

    res = run_bass_kernel_spmd(nc, in_maps, core_ids=list(range(8)), trace=_trace)
    LAST_RESULTS = res

    out = np.empty((B, T, M), dtype=np.float32)
    for b in range(B):
        acc = res.results[4 * b]["r"].astype(np.float32)
        for g in range(1, G):
            acc = acc + res.results[4 * b + g]["r"].astype(np.float32)
        out[b] = acc
    return out
